# revision 1
# baseline (speedup 1.0000x reference)
"""Causal self-attention Bass/Tile kernel for 8-core TRN2.

Sharding: core c handles batch b = c//4, head-group hg = c%4 (4 heads of 16).
Each core computes a partial output y_c = attn_out_local @ W_out_slice.T of
shape (L, E); the host sums the 4 partials per batch.

Per-core compute layout (L=2048, E=1024, D=64, 4 local heads):
  - QKV projection in fp32r (11-bit-mantissa fp32, full-rate on PE) from
    host-pre-transposed xT (E, L) and weight slices.
  - qT, kT produced transposed [d, l] (d on partitions, head pairs packed
    64+64), v produced natural [l, d].
  - Attention in fp16: S^T = K Q^T via row-packed matmuls (K=64 contraction,
    two heads concurrent), exp on ACT (scale=1/8 fused, no max subtraction:
    scores ~ N(0,1)), causal masking via memset + one [128,128] band mask,
    AV col-packed (out^T accumulated in PSUM), denominators via ones-vector
    matmuls, division via reciprocal + PE broadcast matmul + DVE multiply.
  - Out-projection fp16 from attn_out^T [dh, l] into y (L, E) fp32.
"""

import numpy as np

import concourse.bass as bass
import concourse.mybir as mybir
import concourse.tile as tile
from concourse import bacc

F32 = mybir.dt.float32
F32R = mybir.dt.float32r
F16 = mybir.dt.float16

B, L, H, D = 2, 2048, 16, 64
E = H * D  # 1024
HL = 4  # heads per core
DH = HL * D  # 256, local head dims
KC = E // 128  # 8 contraction chunks for projections
NQ = L // 512  # 4 q-chunks
NL = L // 128  # 16 l-tiles


def round_fp32r(x: np.ndarray) -> np.ndarray:
    """Round fp32 to fp32r (11-bit mantissa, RNE on low 12 bits)."""
    u = np.ascontiguousarray(x, dtype=np.float32).view(np.uint32)
    lsb = (u >> 12) & np.uint32(1)
    u = u + np.uint32(0x7FF) + lsb
    u = u & np.uint32(0xFFFFF000)
    return u.view(np.float32)


def build_kernel(phases=("proj", "attn", "oproj"), reps=1):
    nc = bacc.Bacc("TRN2", target_bir_lowering=False, debug=False, num_devices=8)

    xT = nc.dram_tensor("xT", [E, L], F32R, kind="ExternalInput").ap()
    wqT = nc.dram_tensor("wqT", [E, DH], F32R, kind="ExternalInput").ap()
    wkT = nc.dram_tensor("wkT", [E, DH], F32R, kind="ExternalInput").ap()
    wvT = nc.dram_tensor("wvT", [E, DH], F32R, kind="ExternalInput").ap()
    woT = nc.dram_tensor("woT", [DH, E], F16, kind="ExternalInput").ap()
    tri = nc.dram_tensor("tri", [128, 128], F16, kind="ExternalInput").ap()
    y = nc.dram_tensor("y", [L, E], F32, kind="ExternalOutput").ap()

    with tile.TileContext(nc) as tc:
        with (
            tc.tile_pool(name="big", bufs=1) as big,
            tc.tile_pool(name="qk", bufs=1) as qkp,
            tc.tile_pool(name="tmp", bufs=4) as tmp,
            tc.tile_pool(name="exps", bufs=10) as exps,
            tc.tile_pool(name="ps_s", bufs=2, space="PSUM") as ps_s,
            tc.tile_pool(name="ps_acc", bufs=2, space="PSUM") as ps_acc,
            tc.tile_pool(name="ps_sm", bufs=2, space="PSUM") as ps_sm,
        ):
            # ---- static SBUF tensors ----
            X = big.tile([128, KC, L], F32R, tag="X")
            WQ = big.tile([128, KC, DH], F32R, tag="WQ")
            WK = big.tile([128, KC, DH], F32R, tag="WK")
            WV = big.tile([128, KC, DH], F32R, tag="WV")
            WO = big.tile([128, DH // 128, E], F16, tag="WO")
            QT = qkp.tile([128, HL // 2, L], F16, tag="QT")
            KT = qkp.tile([128, HL // 2, L], F16, tag="KT")
            V = big.tile([128, NL, HL, D], F16, tag="V")
            AOT = big.tile([128, DH // 128, L], F16, tag="AOT")
            CM = big.tile([128, 128], F16, tag="CM")
            ONES = big.tile([128, 64], F16, tag="ONES")
            RT = big.tile([128, NQ * 512], F16, tag="RT")

            nc.vector.memset(ONES[:], 1.0)

            # ---- input DMA (weights first, X by l-chunk) ----
            def dma_x_chunk(n):
                sl = slice(n * 512, (n + 1) * 512)
                if n == 0:
                    for kc in range(KC):
                        nc.sync.dma_start(
                            X[:, kc, sl],
                            xT[kc * 128 : (kc + 1) * 128, sl],
                        )
                else:
                    nc.sync.dma_start(
                        X[:, :, sl], xT[:, sl].rearrange("(o p) l -> p o l", p=128)
                    )

            nc.sync.dma_start(WQ[:], wqT.rearrange("(o p) d -> p o d", p=128))
            nc.sync.dma_start(WK[:], wkT.rearrange("(o p) d -> p o d", p=128))
            dma_x_chunk(0)
            nc.sync.dma_start(WV[:], wvT.rearrange("(o p) d -> p o d", p=128))
            nc.sync.dma_start(CM[:], tri)
            nc.sync.dma_start(WO[:], woT.rearrange("(o p) e -> p o e", p=128))
            dma_x_chunk(1)
            dma_x_chunk(2)
            dma_x_chunk(3)

            # ---- per-stage bodies ----
            def proj_qkT(n):
                sl = slice(n * 512, (n + 1) * 512)
                for w, out_t in ((WQ, QT), (WK, KT)):
                    for m in range(2):
                        p = ps_sm.tile([128, 512], F32, tag="sm", name="pp")
                        for kc in range(KC):
                            nc.tensor.matmul(
                                p[:],
                                lhsT=w[:, kc, m * 128 : (m + 1) * 128],
                                rhs=X[:, kc, sl],
                                start=(kc == 0),
                                stop=(kc == KC - 1),
                            )
                        nc.scalar.copy(out_t[:, m, sl], p[:])

            def pv_block(lt):
                if True:
                    p = ps_acc.tile([128, 512], F32, tag="acc", name="pv")[:, 0:256]
                    for kc in range(KC):
                        nc.tensor.matmul(
                            p[:],
                            lhsT=X[:, kc, lt * 128 : (lt + 1) * 128],
                            rhs=WV[:, kc, :],
                            start=(kc == 0),
                            stop=(kc == KC - 1),
                        )
                    nc.scalar.copy(
                        V[:, lt, :, :], p[:].rearrange("p (h d) -> p h d", d=D)
                    )

            def proj_v(n):
                for lt in range(4 * n, 4 * n + 4):
                    pv_block(lt)

            def attn(j, fillers=(), end_fillers=()):
                fillers = list(fillers)
                end_fillers = list(end_fillers)
                qsl = slice(j * 512, (j + 1) * 512)
                for pr in range(HL // 2):  # head pair
                    acc = ps_acc.tile([128, 512], F32, tag="acc", name="acc")
                    sEx = tmp.tile([128, 1024], F16, tag="sEx", name="sEx")
                    nlk = 4 * j + 4
                    prev = None  # (ex, lk, c0) pending AV
                    for lk in range(nlk):
                        m = lk - 4 * j  # >= 0 on diagonal tiles
                        c0 = 128 * m if m > 0 else 0  # first valid column
                        csl = slice(c0, 512)
                        S = ps_s.tile([128, 1024], F32, tag="S", name="S")
                        ex = exps.tile([128, 1024], F16, tag="ex", name="ex")
                        for h2 in range(2):
                            hb = slice(h2 * 64, h2 * 64 + 64)
                            nc.tensor.matmul(
                                S[:, h2 * 512 + c0 : (h2 + 1) * 512],
                                lhsT=KT[hb, pr, lk * 128 : (lk + 1) * 128],
                                rhs=QT[hb, pr, qsl][:, csl],
                                start=True,
                                stop=True,
                            )
                        svw = S[:].rearrange("p (t q) -> p t q", t=2)[:, :, csl]
                        evw = ex[:].rearrange("p (t q) -> p t q", t=2)[:, :, csl]
                        nc.scalar.activation(
                            evw, svw, mybir.ActivationFunctionType.Exp, scale=0.125
                        )
                        if m >= 0:
                            bvw = ex[:].rearrange("p (t q) -> p t q", t=2)[
                                :, :, c0 : c0 + 128
                            ]
                            nc.vector.tensor_mul(
                                bvw, bvw, CM[:, None, :].to_broadcast([128, 2, 128])
                            )
                        if lk == 0:
                            nc.gpsimd.tensor_copy(sEx[:], ex[:])
                        else:
                            swv = sEx[:].rearrange("p (t q) -> p t q", t=2)[:, :, csl]
                            nc.vector.tensor_add(swv, swv, evw)
                        if fillers:
                            fillers.pop(0)()
                        if prev is not None:
                            pex, plk, pc0 = prev
                            for h2 in range(2):
                                nc.tensor.matmul(
                                    acc[h2 * 64 : h2 * 64 + 64, pc0:512],
                                    lhsT=V[:, plk, pr * 2 + h2, :],
                                    rhs=pex[:, h2 * 512 + pc0 : (h2 + 1) * 512],
                                    start=(plk == 0),
                                    stop=False,
                                    skip_group_check=True,
                                )
                        prev = (ex, lk, c0)
                    pex, plk, pc0 = prev
                    for h2 in range(2):
                        nc.tensor.matmul(
                            acc[h2 * 64 : h2 * 64 + 64, pc0:512],
                            lhsT=V[:, plk, pr * 2 + h2, :],
                            rhs=pex[:, h2 * 512 + pc0 : (h2 + 1) * 512],
                            start=(plk == 0),
                            stop=True,
                            skip_group_check=True,
                        )
                    # denominators -> division: recip -> broadcast matmul -> multiply
                    pdf = [
                        ps_sm.tile([128, 512], F32, tag="sm", name="pdf")
                        for _ in range(2)
                    ]
                    for h2 in range(2):
                        nc.tensor.matmul(
                            pdf[h2][0:1, :],
                            lhsT=ONES[:, 0:1],
                            rhs=sEx[:, h2 * 512 : (h2 + 1) * 512],
                            start=True,
                            stop=True,
                        )
                    if end_fillers:
                        end_fillers.pop(0)()
                    bc = ps_sm.tile([128, 512], F32, tag="sm", name="bc")
                    for h2 in range(2):
                        h = pr * 2 + h2
                        rsl = slice(h * 512, h * 512 + 512)
                        with nc.allow_low_precision(reason="fp16 softmax denominators"):
                            nc.vector.reciprocal(RT[0:1, rsl], pdf[h2][0:1, :])
                        nc.tensor.matmul(
                            bc[h2 * 64 : h2 * 64 + 64, :],
                            lhsT=ONES[0:1, :],
                            rhs=RT[0:1, rsl],
                            start=True,
                            stop=True,
                        )
                    bcs = tmp.tile([128, 512], F32, tag="bcs")
                    nc.vector.tensor_copy(bcs[:], bc[:])
                    nc.vector.tensor_mul(AOT[:, pr, qsl], acc[:], bcs[:])

            def oproj_block(lt):
                if True:
                    ysb = tmp.tile([128, 1024], F32, tag="ysb")
                    for ec in range(2):
                        p = ps_sm.tile([128, 512], F32, tag="sm", name="py")
                        for c in range(DH // 128):
                            nc.tensor.matmul(
                                p[:],
                                lhsT=AOT[:, c, lt * 128 : (lt + 1) * 128],
                                rhs=WO[:, c, ec * 512 : (ec + 1) * 512],
                                start=(c == 0),
                                stop=(c == DH // 128 - 1),
                            )
                        nc.any.tensor_copy(ysb[:, ec * 512 : (ec + 1) * 512], p[:])
                    nc.sync.dma_start(
                        y[lt * 128 : (lt + 1) * 128, :], ysb[:]
                    )

            def oproj(st):
                for lt in range(4 * st, 4 * st + 4):
                    oproj_block(lt)

            # ---- phases: proj staggered one chunk ahead of attention ----
            for _rep in range(reps):
                if "proj" in phases:
                    proj_qkT(0)
                    proj_v(0)
                for st in range(NQ):
                    if "proj" in phases and st + 1 < NQ:
                        proj_qkT(st + 1)
                        proj_v(st + 1)
                    if "attn" in phases:
                        attn(st)
                        if "oproj" in phases:
                            oproj(st)
    nc.compile()
    return nc


def host_shard(net_in, W_qkv, W_out):
    """Full inputs -> list of 8 per-core input dicts."""
    tri = (np.arange(128)[None, :] >= np.arange(128)[:, None]).astype(np.float16)
    in_maps = []
    for c in range(8):
        b, hg = divmod(c, 4)
        sl = slice(hg * DH, (hg + 1) * DH)
        in_maps.append(
            {
                "xT": round_fp32r(net_in[b].T),
                "wqT": round_fp32r(W_qkv[0 * E :][sl, :].T),
                "wkT": round_fp32r(W_qkv[1 * E :][sl, :].T),
                "wvT": round_fp32r(W_qkv[2 * E :][sl, :].T),
                "woT": np.ascontiguousarray(W_out[:, sl].T).astype(np.float16),
                "tri": tri,
            }
        )
    return in_maps


def host_unshard(results):
    """8 per-core result dicts -> full (B, L, E) output."""
    out = np.zeros((B, L, E), dtype=np.float32)
    for c in range(8):
        b = c // 4
        out[b] += results[c]["y"]
    return out


_NC_CACHE = {}


def kernel(net_in, W_qkv, W_out):
    """Full inputs -> full (B, L, E) output, computed on 8 TRN2 NeuronCores."""
    net_in = np.ascontiguousarray(np.asarray(net_in, dtype=np.float32))
    W_qkv = np.ascontiguousarray(np.asarray(W_qkv, dtype=np.float32))
    W_out = np.ascontiguousarray(np.asarray(W_out, dtype=np.float32))

    if "nc" not in _NC_CACHE:
        _NC_CACHE["nc"] = build_kernel()
    nc = _NC_CACHE["nc"]

    in_maps = host_shard(net_in, W_qkv, W_out)
    from concourse import bass_utils

    res = bass_utils.run_bass_kernel_spmd(nc, in_maps, core_ids=list(range(8)))
    return host_unshard(res.results)

